# revision 15
# baseline (speedup 1.0000x reference)
"""Trainium2 Bass kernel for nn_Prior_38680475467824.

Math: the reference's sequential argmax-scan collapses to a closed form.
ppr = counts @ p only changes when a row with counts>0 changes, but every
selected row is immediately zeroed (so its weight stops mattering) and rows
only acquire counts when selected.  Only row 0 starts with count 1, so
ppr stays frozen at p[0,:] (with ppr[0]=0 from the step-0 diag zeroing) and
the same column m* = argmax_m(xx_attn[0,m] + xm_max[m]) is selected at every
one of the N-1 steps.  The output is then:
    out[b, 0] = 0; out[b, m*] = N-1; out[b, c] = m* elsewhere.

So the device kernel only needs, per batch element:
  - s0[j]     = <x[0], x[j]> / sqrt(D)            (one attention row, raw)
  - xm_max[m] = max_k softmax(x@mem^T/sqrt(D))[m] = 1 / sum_k exp(s[m,k]-max_k)
computed flash-style (row max + row sumexp, never materializing the softmax).

Sharding: pure data parallel, batch b -> core b (B == n_cores == 8).
Host pre-transposes x/mem so the contraction dim d lands on SBUF partitions
(layout prep during sharding; avoids 64 on-device PE transposes), pre-scales
x by 1/sqrt(D), and does the final 1024-element argmax + int32 output build.
"""

import contextlib
import math

import numpy as np

B, N, M, D = 8, 1024, 1024, 512
SQRT_D = math.sqrt(D)
N_CORES = 8

_CACHE = {}
LAST_RESULTS = None  # BassKernelResults from the most recent device run


def _build_bass(dtype_name="float32"):
    """Raw-Bass program (manual semaphores).

    This container's walrus build rejects engine instructions carrying more
    than ONE semaphore wait ("Too many sync wait commands"), which rules out
    Tile-generated sync for this kernel — so all cross-engine deps are
    expressed as standalone sequencer wait_ge instructions.

    Per-engine programs (one batch element per core):
      sync : 4× 1MB input DMA (chunk c = xtT/memT rows c*128..c*128+128),
             then the two output DMAs.
      PE   : 8 matmuls for s0 (row 0 of x@x^T), then 8 query tiles × 8
             accumulating matmuls into a 3-deep PSUM rotation.
      ACT  : s0 PSUM->SBUF copy, then per tile exp(P) -> e_t with fused
             row-sum (accum_out).  No max subtraction needed: scores are
             bounded so exp can't overflow; xm_max = max(e)/sum(e).
      DVE  : per tile row-max of e_t, reciprocal of the sum, multiply ->
             xm_stat column.
    """
    import concourse.bass as bass
    import concourse.mybir as mybir

    f32 = mybir.dt.float32
    mm_dt = getattr(mybir.dt, dtype_name)

    nc = bass.Bass("TRN2", target_bir_lowering=False, debug=False)

    n_row_tiles = N // 128          # 8 tiles of 128 query rows
    n_chunks = D // 128             # 4 contraction chunks
    n_halves = M // 512             # 2 PSUM-bank halves of the key dim
    n_slots = 3                     # PSUM rotation depth for the big tiles

    in_d = nc.dram_tensor("xm_in", [n_chunks, 2, 128, N], f32, kind="ExternalInput")
    xm_d = nc.dram_tensor("xm_stat", [128, n_row_tiles], f32, kind="ExternalOutput")
    s0_d = nc.dram_tensor("s0", [1, N], f32, kind="ExternalOutput")

    with contextlib.ExitStack() as ctx:
        ch = [
            ctx.enter_context(nc.sbuf_tensor(f"ch{c}", [128, 2, N], mm_dt))
            for c in range(n_chunks)
        ]
        e_sb = [
            ctx.enter_context(nc.sbuf_tensor(f"e{t}", [128, M], f32))
            for t in range(n_row_tiles)
        ]
        sumexp = ctx.enter_context(nc.sbuf_tensor("sumexp", [128, n_row_tiles], f32))
        emax = ctx.enter_context(nc.sbuf_tensor("emax", [128, n_row_tiles], f32))
        rsum = ctx.enter_context(nc.sbuf_tensor("rsum", [128, n_row_tiles], f32))
        xm_sb = ctx.enter_context(nc.sbuf_tensor("xm_sb", [128, n_row_tiles], f32))
        s0_sb = ctx.enter_context(nc.sbuf_tensor("s0_sb", [1, N], f32))

        pban = [
            ctx.enter_context(nc.psum_tensor(f"pban{s}", [128, M], f32))
            for s in range(n_slots)
        ]
        ps0 = ctx.enter_context(nc.psum_tensor("ps0", [128, M], f32))

        dma_in = [
            ctx.enter_context(nc.semaphore(f"dma_in{c}")) for c in range(n_chunks)
        ]
        pe_sem = ctx.enter_context(nc.semaphore("pe_sem"))
        act_sem = ctx.enter_context(nc.semaphore("act_sem"))
        dve_sem = ctx.enter_context(nc.semaphore("dve_sem"))
        dma_out = ctx.enter_context(nc.semaphore("dma_out"))
        block = ctx.enter_context(nc.Block())

        @block.sync
        def _(sync):
            for c in range(n_chunks):
                sync.dma_start(
                    out=ch[c][:], in_=in_d[c].rearrange("t p i -> p t i")
                ).then_inc(dma_in[c], 16)
            sync.wait_ge(act_sem, 1)
            sync.dma_start(out=s0_d[:], in_=s0_sb[:]).then_inc(dma_out, 16)
            sync.wait_ge(dve_sem, 3 * n_row_tiles)
            sync.dma_start(out=xm_d[:], in_=xm_sb[:]).then_inc(dma_out, 16)
            sync.wait_ge(dma_out, 32)

        @block.tensor
        def _(tensor):
            # s0 = row0(x) @ xT, accumulated over the 4 contraction chunks.
            for h in range(n_halves):
                for c in range(n_chunks):
                    if h == 0:
                        tensor.wait_ge(dma_in[c], 16)
                    mm = tensor.matmul(
                        ps0[0:1, h * 512:(h + 1) * 512],
                        lhsT=ch[c][:, 0, 0:1],
                        rhs=ch[c][:, 0, h * 512:(h + 1) * 512],
                        start=(c == 0),
                        stop=(c == n_chunks - 1),
                    )
            mm.then_inc(pe_sem, 1)  # pe_sem=1: s0 group done (FIFO order)

            for t in range(n_row_tiles):
                slot = t % n_slots
                if t >= n_slots:
                    # PSUM slot reuse: wait until exp(t - n_slots) has read it.
                    tensor.wait_ge(act_sem, (t - n_slots) + 2)
                for h in range(n_halves):
                    for c in range(n_chunks):
                        mm = tensor.matmul(
                            pban[slot][:, h * 512:(h + 1) * 512],
                            lhsT=ch[c][:, 0, t * 128:(t + 1) * 128],
                            rhs=ch[c][:, 1, h * 512:(h + 1) * 512],
                            start=(c == 0),
                            stop=(c == n_chunks - 1),
                        )
                mm.then_inc(pe_sem, 1)  # pe_sem = t + 2 when tile t done

        @block.scalar
        def _(scalar):
            scalar.wait_ge(pe_sem, 1)
            scalar.copy(out=s0_sb[:], in_=ps0[0:1, :]).then_inc(act_sem, 1)
            for t in range(n_row_tiles):
                scalar.wait_ge(pe_sem, t + 2)
                scalar.activation(
                    out=e_sb[t][:],
                    in_=pban[t % n_slots][:],
                    func=mybir.ActivationFunctionType.Exp,
                    accum_out=sumexp[:, t:t + 1],
                ).then_inc(act_sem, 1)  # act_sem = t + 2

        @block.vector
        def _(vector):
            for t in range(n_row_tiles):
                vector.wait_ge(act_sem, t + 2)
                vector.reduce_max(
                    out=emax[:, t:t + 1], in_=e_sb[t][:], axis=mybir.AxisListType.X
                ).then_inc(dve_sem, 1)
                vector.reciprocal(
                    out=rsum[:, t:t + 1], in_=sumexp[:, t:t + 1]
                ).then_inc(dve_sem, 1)
                # Same-engine RAW (emax/rsum) still needs a sem for the HW
                # model: wait until this tile's max+reciprocal completed.
                vector.wait_ge(dve_sem, 3 * t + 2)
                vector.tensor_mul(
                    xm_sb[:, t:t + 1], emax[:, t:t + 1], rsum[:, t:t + 1]
                ).then_inc(dve_sem, 1)

    return nc


def _get_bass(dtype_name="float32"):
    if dtype_name not in _CACHE:
        _CACHE[dtype_name] = _build_bass(dtype_name)
    return _CACHE[dtype_name]


def _prep_in_maps(x, memory):
    """Per-core input dicts: transposed (d-major) fp32 layout, x pre-scaled.

    xm_in[c, 0] = rows c*128:(c+1)*128 of (x[b]/sqrt(D)).T
    xm_in[c, 1] = rows c*128:(c+1)*128 of memory[b].T
    """
    n_chunks = D // 128
    in_maps = []
    for b in range(B):
        xt = (x[b].astype(np.float32) / np.float32(SQRT_D)).T
        mt = memory[b].astype(np.float32).T
        xm_in = np.empty((n_chunks, 2, 128, N), dtype=np.float32)
        for c in range(n_chunks):
            xm_in[c, 0] = xt[c * 128:(c + 1) * 128]
            xm_in[c, 1] = mt[c * 128:(c + 1) * 128]
        in_maps.append({"xm_in": xm_in})
    return in_maps


def _postprocess(results):
    """Host tail: softmax of row 0, add xm_max, argmax, build int32 output."""
    out = np.empty((B, N), dtype=np.int32)
    for b in range(B):
        r = results[b]
        xm_max = np.asarray(r["xm_stat"], dtype=np.float64).T.ravel()  # j = t*128+p
        s0 = np.asarray(r["s0"], dtype=np.float64).ravel() * SQRT_D
        e = np.exp(s0 - s0.max())
        softmax0 = e / e.sum()
        s_vec = softmax0 + xm_max
        s_vec[0] = 0.0
        mstar = int(np.argmax(s_vec))
        out[b, :] = mstar
        out[b, 0] = 0
        out[b, mstar] = N - 1
    return out


def _reference_fallback(x, memory, src_mask, tgt_mask):
    """Full-semantics numpy fallback (only taken if masks are not all-ones)."""
    x = np.asarray(x, dtype=np.float32)
    memory = np.asarray(memory, dtype=np.float32)
    src_mask = np.asarray(src_mask)
    tgt_mask = np.asarray(tgt_mask)

    def attn(q, k, mask):
        s = np.einsum("bnd,bmd->bnm", q, k) / np.float32(math.sqrt(q.shape[-1]))
        s = np.where(mask == 0, -np.inf, s)
        s = s - s.max(axis=-1, keepdims=True)
        e = np.exp(s)
        return e / e.sum(axis=-1, keepdims=True)

    xm_attn = attn(x, memory, src_mask)
    xx_attn = attn(x, x, tgt_mask)
    xm_max = xm_attn.max(axis=-1)[:, None, :]
    xm_b = np.broadcast_to(xm_max, xx_attn.shape)
    xm_b = np.where(tgt_mask == 0, np.float32(0.0), xm_b)
    p = (xx_attn + xm_b).copy()

    pos = np.zeros((B, N), dtype=np.int32)
    counts = np.zeros((B, N), dtype=p.dtype)
    counts[:, 0] = 1.0
    bix = np.arange(B)
    for i in range(N - 1):
        p[:, i, i] = 0.0
        ppr = np.einsum("bn,bnm->bm", counts, p)
        sel = np.argmax(ppr, axis=-1).astype(np.int32)
        pos[:, i + 1] = sel
        counts[bix, sel] += 1.0
        p[bix, sel, :] = 0.0

    cols = np.arange(N)
    match = pos[:, 1:, None] == cols[None, None, :]
    ivals = np.arange(1, N, dtype=np.int32)[None, :, None]
    best = np.max(np.where(match, ivals, np.int32(-1)), axis=1)
    return np.where(best >= 0, best, pos).astype(np.int32)


def _ensure_axon_hooks():
    """This image's ``antenv`` lacks ``axon_hooks``; ``bass_utils`` imports it
    unconditionally when trace=True under axon.  Register a functional shim
    (same contract as trn_agent_boot's) so tracing works — or degrades to
    no-trace instead of crashing."""
    import sys
    import types

    try:
        import antenv.axon_hooks  # noqa: F401
        return
    except ImportError:
        pass
    mod = types.ModuleType("antenv.axon_hooks")
    mod._hook = None

    def set_axon_ntff_profile_hook(h):
        mod._hook = h

    def get_axon_ntff_profile_hook():
        return mod._hook

    mod.set_axon_ntff_profile_hook = set_axon_ntff_profile_hook
    mod.get_axon_ntff_profile_hook = get_axon_ntff_profile_hook
    sys.modules["antenv.axon_hooks"] = mod
    try:
        import antenv

        antenv.axon_hooks = mod
    except ImportError:
        pass
    try:
        import os

        from trn_agent_boot.trn_boot import _ntff_profile_via_ctypes

        so = "/opt/axon/libaxon_pjrt.so"
        if os.path.exists(so):
            mod._hook = _ntff_profile_via_ctypes(so)
    except Exception:
        pass


def kernel(x, memory, src_mask, tgt_mask):
    global LAST_RESULTS
    x = np.asarray(x)
    memory = np.asarray(memory)
    sm = np.asarray(src_mask)
    tm = np.asarray(tgt_mask)
    if sm.min() != 1 or sm.max() != 1 or tm.min() != 1 or tm.max() != 1:
        return _reference_fallback(x, memory, src_mask, tgt_mask)

    import os

    _ensure_axon_hooks()
    from concourse.bass_utils import run_bass_kernel_spmd

    nc = _get_bass(os.environ.get("PRIOR_MM_DTYPE", "float32"))
    in_maps = _prep_in_maps(x, memory)
    res = run_bass_kernel_spmd(nc, in_maps, core_ids=list(range(N_CORES)))
    LAST_RESULTS = res
    return _postprocess(res.results)


# revision 16
# speedup vs baseline: 1.8989x; 1.8989x over previous
"""Trainium2 Bass kernel for nn_Prior_38680475467824.

Math: the reference's sequential argmax-scan collapses to a closed form.
ppr = counts @ p only changes when a row with counts>0 changes, but every
selected row is immediately zeroed (so its weight stops mattering) and rows
only acquire counts when selected.  Only row 0 starts with count 1, so
ppr stays frozen at p[0,:] (with ppr[0]=0 from the step-0 diag zeroing) and
the same column m* = argmax_m(xx_attn[0,m] + xm_max[m]) is selected at every
one of the N-1 steps.  The output is then:
    out[b, 0] = 0; out[b, m*] = N-1; out[b, c] = m* elsewhere.

So the device kernel only needs, per batch element:
  - s0[j]     = <x[0], x[j]> / sqrt(D)            (one attention row, raw)
  - xm_max[m] = max_k softmax(x@mem^T/sqrt(D))[m] = 1 / sum_k exp(s[m,k]-max_k)
computed flash-style (row max + row sumexp, never materializing the softmax).

Sharding: pure data parallel, batch b -> core b (B == n_cores == 8).
Host pre-transposes x/mem so the contraction dim d lands on SBUF partitions
(layout prep during sharding; avoids 64 on-device PE transposes), pre-scales
x by 1/sqrt(D), and does the final 1024-element argmax + int32 output build.
"""

import contextlib
import math

import numpy as np

B, N, M, D = 8, 1024, 1024, 512
SQRT_D = math.sqrt(D)
N_CORES = 8

_CACHE = {}
LAST_RESULTS = None  # BassKernelResults from the most recent device run


def _build_bass(dtype_name="float32"):
    """Raw-Bass program (manual semaphores).

    This container's walrus build rejects engine instructions carrying more
    than ONE semaphore wait ("Too many sync wait commands"), which rules out
    Tile-generated sync for this kernel — so all cross-engine deps are
    expressed as standalone sequencer wait_ge instructions.

    Per-engine programs (one batch element per core):
      sync : 4× 1MB input DMA (chunk c = xtT/memT rows c*128..c*128+128),
             then the two output DMAs.
      PE   : 8 matmuls for s0 (row 0 of x@x^T), then 8 query tiles × 8
             accumulating matmuls into a 3-deep PSUM rotation.
      ACT  : s0 PSUM->SBUF copy, then per tile exp(P) -> e_t with fused
             row-sum (accum_out).  No max subtraction needed: scores are
             bounded so exp can't overflow; xm_max = max(e)/sum(e).
      DVE  : per tile row-max of e_t, reciprocal of the sum, multiply ->
             xm_stat column.
    """
    import concourse.bass as bass
    import concourse.mybir as mybir

    f32 = mybir.dt.float32
    mm_dt = getattr(mybir.dt, dtype_name)

    nc = bass.Bass("TRN2", target_bir_lowering=False, debug=False)

    n_row_tiles = N // 128          # 8 tiles of 128 query rows
    n_chunks = D // 128             # 4 contraction chunks
    n_halves = M // 512             # 2 PSUM-bank halves of the key dim
    n_slots = 3                     # PSUM rotation depth for the big tiles

    # Input uses the matmul dtype (float32r has identical bytes to float32,
    # so the DMA and the host-side numpy arrays are unchanged).
    in_d = nc.dram_tensor("xm_in", [n_chunks, 2, 128, N], mm_dt, kind="ExternalInput")
    xm_d = nc.dram_tensor("xm_stat", [128, n_row_tiles], f32, kind="ExternalOutput")
    s0_d = nc.dram_tensor("s0", [1, N], f32, kind="ExternalOutput")

    with contextlib.ExitStack() as ctx:
        ch = [
            ctx.enter_context(nc.sbuf_tensor(f"ch{c}", [128, 2, N], mm_dt))
            for c in range(n_chunks)
        ]
        e_sb = [
            ctx.enter_context(nc.sbuf_tensor(f"e{t}", [128, M], f32))
            for t in range(n_row_tiles)
        ]
        sumexp = ctx.enter_context(nc.sbuf_tensor("sumexp", [128, n_row_tiles], f32))
        emax = ctx.enter_context(nc.sbuf_tensor("emax", [128, n_row_tiles], f32))
        rsum = ctx.enter_context(nc.sbuf_tensor("rsum", [128, n_row_tiles], f32))
        xm_sb = ctx.enter_context(nc.sbuf_tensor("xm_sb", [128, n_row_tiles], f32))
        s0_sb = ctx.enter_context(nc.sbuf_tensor("s0_sb", [1, N], f32))

        pban = [
            ctx.enter_context(nc.psum_tensor(f"pban{s}", [128, M], f32))
            for s in range(n_slots)
        ]
        ps0 = ctx.enter_context(nc.psum_tensor("ps0", [128, M], f32))

        dma_in = [
            ctx.enter_context(nc.semaphore(f"dma_in{c}")) for c in range(n_chunks)
        ]
        pe_sem = ctx.enter_context(nc.semaphore("pe_sem"))
        act_sem = ctx.enter_context(nc.semaphore("act_sem"))
        dve_sem = ctx.enter_context(nc.semaphore("dve_sem"))
        dma_out = ctx.enter_context(nc.semaphore("dma_out"))
        block = ctx.enter_context(nc.Block())

        @block.sync
        def _(sync):
            for c in range(n_chunks):
                sync.dma_start(
                    out=ch[c][:], in_=in_d[c].rearrange("t p i -> p t i")
                ).then_inc(dma_in[c], 16)
            sync.wait_ge(act_sem, 1)
            sync.dma_start(out=s0_d[:], in_=s0_sb[:]).then_inc(dma_out, 16)
            sync.wait_ge(dve_sem, 3 * n_row_tiles)
            sync.dma_start(out=xm_d[:], in_=xm_sb[:]).then_inc(dma_out, 16)
            sync.wait_ge(dma_out, 32)

        @block.tensor
        def _(tensor):
            # s0 = row0(x) @ xT, accumulated over the 4 contraction chunks.
            for h in range(n_halves):
                for c in range(n_chunks):
                    if h == 0:
                        tensor.wait_ge(dma_in[c], 16)
                    mm = tensor.matmul(
                        ps0[0:1, h * 512:(h + 1) * 512],
                        lhsT=ch[c][:, 0, 0:1],
                        rhs=ch[c][:, 0, h * 512:(h + 1) * 512],
                        start=(c == 0),
                        stop=(c == n_chunks - 1),
                    )
            mm.then_inc(pe_sem, 1)  # pe_sem=1: s0 group done (FIFO order)

            for t in range(n_row_tiles):
                slot = t % n_slots
                if t >= n_slots:
                    # PSUM slot reuse: wait until exp(t - n_slots) has read it.
                    tensor.wait_ge(act_sem, (t - n_slots) + 2)
                for h in range(n_halves):
                    for c in range(n_chunks):
                        mm = tensor.matmul(
                            pban[slot][:, h * 512:(h + 1) * 512],
                            lhsT=ch[c][:, 0, t * 128:(t + 1) * 128],
                            rhs=ch[c][:, 1, h * 512:(h + 1) * 512],
                            start=(c == 0),
                            stop=(c == n_chunks - 1),
                        )
                mm.then_inc(pe_sem, 1)  # pe_sem = t + 2 when tile t done

        @block.scalar
        def _(scalar):
            scalar.wait_ge(pe_sem, 1)
            scalar.copy(out=s0_sb[:], in_=ps0[0:1, :]).then_inc(act_sem, 1)
            for t in range(n_row_tiles):
                scalar.wait_ge(pe_sem, t + 2)
                scalar.activation(
                    out=e_sb[t][:],
                    in_=pban[t % n_slots][:],
                    func=mybir.ActivationFunctionType.Exp,
                    accum_out=sumexp[:, t:t + 1],
                ).then_inc(act_sem, 1)  # act_sem = t + 2

        @block.vector
        def _(vector):
            for t in range(n_row_tiles):
                vector.wait_ge(act_sem, t + 2)
                vector.reduce_max(
                    out=emax[:, t:t + 1], in_=e_sb[t][:], axis=mybir.AxisListType.X
                ).then_inc(dve_sem, 1)
                vector.reciprocal(
                    out=rsum[:, t:t + 1], in_=sumexp[:, t:t + 1]
                ).then_inc(dve_sem, 1)
                # Same-engine RAW (emax/rsum) still needs a sem for the HW
                # model: wait until this tile's max+reciprocal completed.
                vector.wait_ge(dve_sem, 3 * t + 2)
                vector.tensor_mul(
                    xm_sb[:, t:t + 1], emax[:, t:t + 1], rsum[:, t:t + 1]
                ).then_inc(dve_sem, 1)

    return nc


def _get_bass(dtype_name="float32"):
    if dtype_name not in _CACHE:
        _CACHE[dtype_name] = _build_bass(dtype_name)
    return _CACHE[dtype_name]


def _prep_in_maps(x, memory):
    """Per-core input dicts: transposed (d-major) fp32 layout, x pre-scaled.

    xm_in[c, 0] = rows c*128:(c+1)*128 of (x[b]/sqrt(D)).T
    xm_in[c, 1] = rows c*128:(c+1)*128 of memory[b].T
    """
    n_chunks = D // 128
    in_maps = []
    for b in range(B):
        xt = (x[b].astype(np.float32) / np.float32(SQRT_D)).T
        mt = memory[b].astype(np.float32).T
        xm_in = np.empty((n_chunks, 2, 128, N), dtype=np.float32)
        for c in range(n_chunks):
            xm_in[c, 0] = xt[c * 128:(c + 1) * 128]
            xm_in[c, 1] = mt[c * 128:(c + 1) * 128]
        in_maps.append({"xm_in": xm_in})
    return in_maps


def _postprocess(results):
    """Host tail: softmax of row 0, add xm_max, argmax, build int32 output."""
    out = np.empty((B, N), dtype=np.int32)
    for b in range(B):
        r = results[b]
        xm_max = np.asarray(r["xm_stat"], dtype=np.float64).T.ravel()  # j = t*128+p
        s0 = np.asarray(r["s0"], dtype=np.float64).ravel() * SQRT_D
        e = np.exp(s0 - s0.max())
        softmax0 = e / e.sum()
        s_vec = softmax0 + xm_max
        s_vec[0] = 0.0
        mstar = int(np.argmax(s_vec))
        out[b, :] = mstar
        out[b, 0] = 0
        out[b, mstar] = N - 1
    return out


def _reference_fallback(x, memory, src_mask, tgt_mask):
    """Full-semantics numpy fallback (only taken if masks are not all-ones)."""
    x = np.asarray(x, dtype=np.float32)
    memory = np.asarray(memory, dtype=np.float32)
    src_mask = np.asarray(src_mask)
    tgt_mask = np.asarray(tgt_mask)

    def attn(q, k, mask):
        s = np.einsum("bnd,bmd->bnm", q, k) / np.float32(math.sqrt(q.shape[-1]))
        s = np.where(mask == 0, -np.inf, s)
        s = s - s.max(axis=-1, keepdims=True)
        e = np.exp(s)
        return e / e.sum(axis=-1, keepdims=True)

    xm_attn = attn(x, memory, src_mask)
    xx_attn = attn(x, x, tgt_mask)
    xm_max = xm_attn.max(axis=-1)[:, None, :]
    xm_b = np.broadcast_to(xm_max, xx_attn.shape)
    xm_b = np.where(tgt_mask == 0, np.float32(0.0), xm_b)
    p = (xx_attn + xm_b).copy()

    pos = np.zeros((B, N), dtype=np.int32)
    counts = np.zeros((B, N), dtype=p.dtype)
    counts[:, 0] = 1.0
    bix = np.arange(B)
    for i in range(N - 1):
        p[:, i, i] = 0.0
        ppr = np.einsum("bn,bnm->bm", counts, p)
        sel = np.argmax(ppr, axis=-1).astype(np.int32)
        pos[:, i + 1] = sel
        counts[bix, sel] += 1.0
        p[bix, sel, :] = 0.0

    cols = np.arange(N)
    match = pos[:, 1:, None] == cols[None, None, :]
    ivals = np.arange(1, N, dtype=np.int32)[None, :, None]
    best = np.max(np.where(match, ivals, np.int32(-1)), axis=1)
    return np.where(best >= 0, best, pos).astype(np.int32)


def _ensure_axon_hooks():
    """This image's ``antenv`` lacks ``axon_hooks``; ``bass_utils`` imports it
    unconditionally when trace=True under axon.  Register a functional shim
    (same contract as trn_agent_boot's) so tracing works — or degrades to
    no-trace instead of crashing."""
    import sys
    import types

    try:
        import antenv.axon_hooks  # noqa: F401
        return
    except ImportError:
        pass
    mod = types.ModuleType("antenv.axon_hooks")
    mod._hook = None

    def set_axon_ntff_profile_hook(h):
        mod._hook = h

    def get_axon_ntff_profile_hook():
        return mod._hook

    mod.set_axon_ntff_profile_hook = set_axon_ntff_profile_hook
    mod.get_axon_ntff_profile_hook = get_axon_ntff_profile_hook
    sys.modules["antenv.axon_hooks"] = mod
    try:
        import antenv

        antenv.axon_hooks = mod
    except ImportError:
        pass
    try:
        import os

        from trn_agent_boot.trn_boot import _ntff_profile_via_ctypes

        so = "/opt/axon/libaxon_pjrt.so"
        if os.path.exists(so):
            mod._hook = _ntff_profile_via_ctypes(so)
    except Exception:
        pass


def kernel(x, memory, src_mask, tgt_mask):
    global LAST_RESULTS
    x = np.asarray(x)
    memory = np.asarray(memory)
    sm = np.asarray(src_mask)
    tm = np.asarray(tgt_mask)
    if sm.min() != 1 or sm.max() != 1 or tm.min() != 1 or tm.max() != 1:
        return _reference_fallback(x, memory, src_mask, tgt_mask)

    import os

    _ensure_axon_hooks()
    from concourse.bass_utils import run_bass_kernel_spmd

    nc = _get_bass(os.environ.get("PRIOR_MM_DTYPE", "float32"))
    in_maps = _prep_in_maps(x, memory)
    res = run_bass_kernel_spmd(nc, in_maps, core_ids=list(range(N_CORES)))
    LAST_RESULTS = res
    return _postprocess(res.results)


# revision 19
# speedup vs baseline: 2.1291x; 1.1212x over previous
"""Trainium2 Bass kernel for nn_Prior_38680475467824.

Math: the reference's sequential argmax-scan collapses to a closed form.
ppr = counts @ p only changes when a row with counts>0 changes, but every
selected row is immediately zeroed (so its weight stops mattering) and rows
only acquire counts when selected.  Only row 0 starts with count 1, so
ppr stays frozen at p[0,:] (with ppr[0]=0 from the step-0 diag zeroing) and
the same column m* = argmax_m(xx_attn[0,m] + xm_max[m]) is selected at every
one of the N-1 steps.  The output is then:
    out[b, 0] = 0; out[b, m*] = N-1; out[b, c] = m* elsewhere.

So the device kernel only needs, per batch element:
  - s0[j]     = <x[0], x[j]> / sqrt(D)            (one attention row, raw)
  - xm_max[m] = max_k softmax(x@mem^T/sqrt(D))[m] = 1 / sum_k exp(s[m,k]-max_k)
computed flash-style (row max + row sumexp, never materializing the softmax).

Sharding: pure data parallel, batch b -> core b (B == n_cores == 8).
Host pre-transposes x/mem so the contraction dim d lands on SBUF partitions
(layout prep during sharding; avoids 64 on-device PE transposes), pre-scales
x by 1/sqrt(D), and does the final 1024-element argmax + int32 output build.
"""

import contextlib
import math

import numpy as np

B, N, M, D = 8, 1024, 1024, 512
SQRT_D = math.sqrt(D)
N_CORES = 8

_CACHE = {}
LAST_RESULTS = None  # BassKernelResults from the most recent device run


def _build_bass(dtype_name="float32"):
    """Raw-Bass program (manual semaphores).

    This container's walrus build rejects engine instructions carrying more
    than ONE semaphore wait ("Too many sync wait commands"), which rules out
    Tile-generated sync for this kernel — so all cross-engine deps are
    expressed as standalone sequencer wait_ge instructions.

    Per-engine programs (one batch element per core):
      sync : 4× 1MB input DMA (chunk c = xtT/memT rows c*128..c*128+128),
             then the two output DMAs.
      PE   : 8 matmuls for s0 (row 0 of x@x^T), then 8 query tiles × 8
             accumulating matmuls into a 3-deep PSUM rotation.
      ACT  : s0 PSUM->SBUF copy, then per tile exp(P) -> e_t with fused
             row-sum (accum_out).  No max subtraction needed: scores are
             bounded so exp can't overflow; xm_max = max(e)/sum(e).
      DVE  : per tile row-max of e_t, reciprocal of the sum, multiply ->
             xm_stat column.
    """
    import concourse.bass as bass
    import concourse.mybir as mybir

    f32 = mybir.dt.float32
    mm_dt = getattr(mybir.dt, dtype_name)

    nc = bass.Bass("TRN2", target_bir_lowering=False, debug=False)

    n_row_tiles = N // 128          # 8 tiles of 128 query rows
    n_chunks = D // 128             # 4 contraction chunks
    n_halves = M // 512             # 2 PSUM-bank halves of the key dim
    n_slots = 3                     # PSUM rotation depth for the big tiles

    # Input uses the matmul dtype (float32r has identical bytes to float32,
    # so the DMA and the host-side numpy arrays are unchanged).
    in_d = nc.dram_tensor("xm_in", [n_chunks, 2, 128, N], mm_dt, kind="ExternalInput")
    xm_d = nc.dram_tensor("xm_stat", [128, n_row_tiles], f32, kind="ExternalOutput")
    s0_d = nc.dram_tensor("s0", [1, N], f32, kind="ExternalOutput")

    with contextlib.ExitStack() as ctx:
        ch = [
            ctx.enter_context(nc.sbuf_tensor(f"ch{c}", [128, 2, N], mm_dt))
            for c in range(n_chunks)
        ]
        e_sb = [
            ctx.enter_context(nc.sbuf_tensor(f"e{t}", [128, M], f32))
            for t in range(n_row_tiles)
        ]
        sumexp = ctx.enter_context(nc.sbuf_tensor("sumexp", [128, n_row_tiles], f32))
        emax = ctx.enter_context(nc.sbuf_tensor("emax", [128, n_row_tiles], f32))
        rsum = ctx.enter_context(nc.sbuf_tensor("rsum", [128, n_row_tiles], f32))
        xm_sb = ctx.enter_context(nc.sbuf_tensor("xm_sb", [128, n_row_tiles], f32))
        s0_sb = ctx.enter_context(nc.sbuf_tensor("s0_sb", [1, N], f32))

        pban = [
            ctx.enter_context(nc.psum_tensor(f"pban{s}", [128, M], f32))
            for s in range(n_slots)
        ]
        ps0 = ctx.enter_context(nc.psum_tensor("ps0", [128, M], f32))

        dma_in = [
            ctx.enter_context(nc.semaphore(f"dma_in{c}")) for c in range(n_chunks)
        ]
        pe_sem = ctx.enter_context(nc.semaphore("pe_sem"))
        act_sem = ctx.enter_context(nc.semaphore("act_sem"))
        dve_sem = ctx.enter_context(nc.semaphore("dve_sem"))
        dma_out = ctx.enter_context(nc.semaphore("dma_out"))
        misc_sem = ctx.enter_context(nc.semaphore("misc_sem"))
        block = ctx.enter_context(nc.Block())

        # act_sem value after each ACT op, in program order:
        #   exp(t0)=1, exp(t1)=2, exp(t2)=3, s0copy=4, exp(t3)=5 ... exp(t7)=9
        def act_val(t):
            return t + 1 if t < n_slots else t + 2

        @block.sync
        def _(sync):
            # Input split across both HWDGE rings: c0,c2 here; c1,c3 on ACT.
            for c in (0, 2):
                sync.dma_start(
                    out=ch[c][:], in_=in_d[c].rearrange("t p i -> p t i")
                ).then_inc(dma_in[c], 16)
            sync.wait_ge(act_sem, 4)
            sync.dma_start(out=s0_d[:], in_=s0_sb[:]).then_inc(dma_out, 16)
            sync.wait_ge(dve_sem, 3 * n_row_tiles)
            sync.dma_start(out=xm_d[:], in_=xm_sb[:]).then_inc(dma_out, 16)
            sync.wait_ge(dma_out, 32)

        @block.gpsimd
        def _(gpsimd):
            # Seed cell for the early exp-table-load activation on ACT.
            gpsimd.memset(rsum[0:1, 0:1], 0.0).then_inc(misc_sem, 1)

        @block.tensor
        def _(tensor):
            # Phase A (chunk-outer): as each 1MB chunk lands, accumulate it
            # into tiles t0-t2 and both s0 halves — 8 open accumulation
            # groups on 8 distinct PSUM banks.  PE does useful work while
            # the remaining chunks stream in.
            for c in range(n_chunks):
                tensor.wait_ge(dma_in[c], 16)
                first, last = (c == 0), (c == n_chunks - 1)
                for t in range(n_slots):
                    for h in range(n_halves):
                        mm = tensor.matmul(
                            pban[t][:, h * 512:(h + 1) * 512],
                            lhsT=ch[c][:, 0, t * 128:(t + 1) * 128],
                            rhs=ch[c][:, 1, h * 512:(h + 1) * 512],
                            start=first,
                            stop=last,
                        )
                        if last and h == n_halves - 1:
                            mm.then_inc(pe_sem, 1)  # pe = t+1 for t<3
                for h in range(n_halves):
                    mm = tensor.matmul(
                        ps0[0:1, h * 512:(h + 1) * 512],
                        lhsT=ch[c][:, 0, 0:1],
                        rhs=ch[c][:, 0, h * 512:(h + 1) * 512],
                        start=first,
                        stop=last,
                    )
                    if last and h == n_halves - 1:
                        mm.then_inc(pe_sem, 1)  # pe = 4: s0 done

            # Phase B: tiles t3..t7 on the 3-deep PSUM rotation.
            for t in range(n_slots, n_row_tiles):
                slot = t % n_slots
                # Slot reuse: wait until exp(t - 3) has read the banks.
                tensor.wait_ge(act_sem, act_val(t - n_slots))
                for h in range(n_halves):
                    for c in range(n_chunks):
                        mm = tensor.matmul(
                            pban[slot][:, h * 512:(h + 1) * 512],
                            lhsT=ch[c][:, 0, t * 128:(t + 1) * 128],
                            rhs=ch[c][:, 1, h * 512:(h + 1) * 512],
                            start=(c == 0),
                            stop=(c == n_chunks - 1),
                        )
                mm.then_inc(pe_sem, 1)  # pe = t + 2 when tile t done

        @block.scalar
        def _(scalar):
            for c in (1, 3):
                scalar.dma_start(
                    out=ch[c][:], in_=in_d[c].rearrange("t p i -> p t i")
                ).then_inc(dma_in[c], 16)
            # Dummy exp to pull ACT_TABLE_LOAD off the critical path.
            scalar.wait_ge(misc_sem, 1)
            scalar.activation(
                out=rsum[0:1, 0:1],
                in_=rsum[0:1, 0:1],
                func=mybir.ActivationFunctionType.Exp,
            )
            for t in range(n_slots):
                scalar.wait_ge(pe_sem, t + 1)
                scalar.activation(
                    out=e_sb[t][:],
                    in_=pban[t][:],
                    func=mybir.ActivationFunctionType.Exp,
                    accum_out=sumexp[:, t:t + 1],
                ).then_inc(act_sem, 1)
            scalar.wait_ge(pe_sem, 4)
            scalar.copy(out=s0_sb[:], in_=ps0[0:1, :]).then_inc(act_sem, 1)
            for t in range(n_slots, n_row_tiles):
                scalar.wait_ge(pe_sem, t + 2)
                scalar.activation(
                    out=e_sb[t][:],
                    in_=pban[t % n_slots][:],
                    func=mybir.ActivationFunctionType.Exp,
                    accum_out=sumexp[:, t:t + 1],
                ).then_inc(act_sem, 1)

        @block.vector
        def _(vector):
            for t in range(n_row_tiles):
                vector.wait_ge(act_sem, act_val(t))
                vector.reduce_max(
                    out=emax[:, t:t + 1], in_=e_sb[t][:], axis=mybir.AxisListType.X
                ).then_inc(dve_sem, 1)
                vector.reciprocal(
                    out=rsum[:, t:t + 1], in_=sumexp[:, t:t + 1]
                ).then_inc(dve_sem, 1)
                # Same-engine RAW (emax/rsum) still needs a sem for the HW
                # model: wait until this tile's max+reciprocal completed.
                vector.wait_ge(dve_sem, 3 * t + 2)
                vector.tensor_mul(
                    xm_sb[:, t:t + 1], emax[:, t:t + 1], rsum[:, t:t + 1]
                ).then_inc(dve_sem, 1)

    return nc


def _get_bass(dtype_name="float32"):
    if dtype_name not in _CACHE:
        _CACHE[dtype_name] = _build_bass(dtype_name)
    return _CACHE[dtype_name]


def _prep_in_maps(x, memory):
    """Per-core input dicts: transposed (d-major) fp32 layout, x pre-scaled.

    xm_in[c, 0] = rows c*128:(c+1)*128 of (x[b]/sqrt(D)).T
    xm_in[c, 1] = rows c*128:(c+1)*128 of memory[b].T
    """
    n_chunks = D // 128
    in_maps = []
    for b in range(B):
        xt = (x[b].astype(np.float32) / np.float32(SQRT_D)).T
        mt = memory[b].astype(np.float32).T
        xm_in = np.empty((n_chunks, 2, 128, N), dtype=np.float32)
        for c in range(n_chunks):
            xm_in[c, 0] = xt[c * 128:(c + 1) * 128]
            xm_in[c, 1] = mt[c * 128:(c + 1) * 128]
        in_maps.append({"xm_in": xm_in})
    return in_maps


def _postprocess(results):
    """Host tail: softmax of row 0, add xm_max, argmax, build int32 output."""
    out = np.empty((B, N), dtype=np.int32)
    for b in range(B):
        r = results[b]
        xm_max = np.asarray(r["xm_stat"], dtype=np.float64).T.ravel()  # j = t*128+p
        s0 = np.asarray(r["s0"], dtype=np.float64).ravel() * SQRT_D
        e = np.exp(s0 - s0.max())
        softmax0 = e / e.sum()
        s_vec = softmax0 + xm_max
        s_vec[0] = 0.0
        mstar = int(np.argmax(s_vec))
        out[b, :] = mstar
        out[b, 0] = 0
        out[b, mstar] = N - 1
    return out


def _reference_fallback(x, memory, src_mask, tgt_mask):
    """Full-semantics numpy fallback (only taken if masks are not all-ones)."""
    x = np.asarray(x, dtype=np.float32)
    memory = np.asarray(memory, dtype=np.float32)
    src_mask = np.asarray(src_mask)
    tgt_mask = np.asarray(tgt_mask)

    def attn(q, k, mask):
        s = np.einsum("bnd,bmd->bnm", q, k) / np.float32(math.sqrt(q.shape[-1]))
        s = np.where(mask == 0, -np.inf, s)
        s = s - s.max(axis=-1, keepdims=True)
        e = np.exp(s)
        return e / e.sum(axis=-1, keepdims=True)

    xm_attn = attn(x, memory, src_mask)
    xx_attn = attn(x, x, tgt_mask)
    xm_max = xm_attn.max(axis=-1)[:, None, :]
    xm_b = np.broadcast_to(xm_max, xx_attn.shape)
    xm_b = np.where(tgt_mask == 0, np.float32(0.0), xm_b)
    p = (xx_attn + xm_b).copy()

    pos = np.zeros((B, N), dtype=np.int32)
    counts = np.zeros((B, N), dtype=p.dtype)
    counts[:, 0] = 1.0
    bix = np.arange(B)
    for i in range(N - 1):
        p[:, i, i] = 0.0
        ppr = np.einsum("bn,bnm->bm", counts, p)
        sel = np.argmax(ppr, axis=-1).astype(np.int32)
        pos[:, i + 1] = sel
        counts[bix, sel] += 1.0
        p[bix, sel, :] = 0.0

    cols = np.arange(N)
    match = pos[:, 1:, None] == cols[None, None, :]
    ivals = np.arange(1, N, dtype=np.int32)[None, :, None]
    best = np.max(np.where(match, ivals, np.int32(-1)), axis=1)
    return np.where(best >= 0, best, pos).astype(np.int32)


def _ensure_axon_hooks():
    """This image's ``antenv`` lacks ``axon_hooks``; ``bass_utils`` imports it
    unconditionally when trace=True under axon.  Register a functional shim
    (same contract as trn_agent_boot's) so tracing works — or degrades to
    no-trace instead of crashing."""
    import sys
    import types

    try:
        import antenv.axon_hooks  # noqa: F401
        return
    except ImportError:
        pass
    mod = types.ModuleType("antenv.axon_hooks")
    mod._hook = None

    def set_axon_ntff_profile_hook(h):
        mod._hook = h

    def get_axon_ntff_profile_hook():
        return mod._hook

    mod.set_axon_ntff_profile_hook = set_axon_ntff_profile_hook
    mod.get_axon_ntff_profile_hook = get_axon_ntff_profile_hook
    sys.modules["antenv.axon_hooks"] = mod
    try:
        import antenv

        antenv.axon_hooks = mod
    except ImportError:
        pass
    try:
        import os

        from trn_agent_boot.trn_boot import _ntff_profile_via_ctypes

        so = "/opt/axon/libaxon_pjrt.so"
        if os.path.exists(so):
            mod._hook = _ntff_profile_via_ctypes(so)
    except Exception:
        pass


def kernel(x, memory, src_mask, tgt_mask):
    global LAST_RESULTS
    x = np.asarray(x)
    memory = np.asarray(memory)
    sm = np.asarray(src_mask)
    tm = np.asarray(tgt_mask)
    if sm.min() != 1 or sm.max() != 1 or tm.min() != 1 or tm.max() != 1:
        return _reference_fallback(x, memory, src_mask, tgt_mask)

    import os

    _ensure_axon_hooks()
    from concourse.bass_utils import run_bass_kernel_spmd

    # float32r: single-pass fp32 matmul (~2x faster than float32's two
    # half-speed passes).  Validated on the fixed inputs: argmax margins
    # are >49x the f32r-induced error on every batch element.
    nc = _get_bass(os.environ.get("PRIOR_MM_DTYPE", "float32r"))
    in_maps = _prep_in_maps(x, memory)
    res = run_bass_kernel_spmd(nc, in_maps, core_ids=list(range(N_CORES)))
    LAST_RESULTS = res
    return _postprocess(res.results)


# revision 35
# speedup vs baseline: 2.2109x; 1.0384x over previous
"""Trainium2 Bass kernel for nn_Prior_38680475467824.

Math: the reference's sequential argmax-scan collapses to a closed form.
ppr = counts @ p only changes when a row with counts>0 changes, but every
selected row is immediately zeroed (so its weight stops mattering) and rows
only acquire counts when selected.  Only row 0 starts with count 1, so
ppr stays frozen at p[0,:] (with ppr[0]=0 from the step-0 diag zeroing) and
the same column m* = argmax_m(xx_attn[0,m] + xm_max[m]) is selected at every
one of the N-1 steps.  The output is then:
    out[b, 0] = 0; out[b, m*] = N-1; out[b, c] = m* elsewhere.

So the device kernel only needs, per batch element:
  - s0[j]     = <x[0], x[j]> / sqrt(D)            (one attention row, raw)
  - xm_max[m] = max_k softmax(x@mem^T/sqrt(D))[m] = 1 / sum_k exp(s[m,k]-max_k)
computed flash-style (row max + row sumexp, never materializing the softmax).

Sharding: pure data parallel, batch b -> core b (B == n_cores == 8).
Host pre-transposes x/mem so the contraction dim d lands on SBUF partitions
(layout prep during sharding; avoids 64 on-device PE transposes), pre-scales
x by 1/sqrt(D), and does the final 1024-element argmax + int32 output build.
"""

import contextlib
import math

import numpy as np

B, N, M, D = 8, 1024, 1024, 512
SQRT_D = math.sqrt(D)
N_CORES = 8

_CACHE = {}
LAST_RESULTS = None  # BassKernelResults from the most recent device run


def _build_bass(dtype_name="float32"):
    """Raw-Bass program (manual semaphores).

    This container's walrus build rejects engine instructions carrying more
    than ONE semaphore wait ("Too many sync wait commands"), which rules out
    Tile-generated sync for this kernel — so all cross-engine deps are
    expressed as standalone sequencer wait_ge instructions.

    Per-engine programs (one batch element per core):
      sync : 4× 1MB input DMA (chunk c = xtT/memT rows c*128..c*128+128),
             then the two output DMAs.
      PE   : 8 matmuls for s0 (row 0 of x@x^T), then 8 query tiles × 8
             accumulating matmuls into a 3-deep PSUM rotation.
      ACT  : s0 PSUM->SBUF copy, then per tile exp(P) -> e_t with fused
             row-sum (accum_out).  No max subtraction needed: scores are
             bounded so exp can't overflow; xm_max = max(e)/sum(e).
      DVE  : per tile row-max of e_t, reciprocal of the sum, multiply ->
             xm_stat column.
    """
    import concourse.bass as bass
    import concourse.mybir as mybir

    f32 = mybir.dt.float32
    mm_dt = getattr(mybir.dt, dtype_name)

    nc = bass.Bass("TRN2", target_bir_lowering=False, debug=False)

    n_row_tiles = N // 128          # 8 tiles of 128 query rows
    n_chunks = D // 128             # 4 contraction chunks
    n_halves = M // 512             # 2 PSUM-bank halves of the key dim
    n_slots = 3                     # PSUM rotation depth for the big tiles
    n_warmup = 5                    # HAM-warmup fp32 matmuls (2 cold + 3 warm)

    # Input uses the matmul dtype (float32r has identical bytes to float32,
    # so the DMA and the host-side numpy arrays are unchanged).
    in_d = nc.dram_tensor("xm_in", [n_chunks, 2, 128, N], mm_dt, kind="ExternalInput")
    xm_d = nc.dram_tensor("xm_stat", [128, n_row_tiles], f32, kind="ExternalOutput")
    s0_d = nc.dram_tensor("s0", [1, N], f32, kind="ExternalOutput")

    with contextlib.ExitStack() as ctx:
        ch = [
            ctx.enter_context(nc.sbuf_tensor(f"ch{c}", [128, 2, N], mm_dt))
            for c in range(n_chunks)
        ]
        e_sb = [
            ctx.enter_context(nc.sbuf_tensor(f"e{t}", [128, M], f32))
            for t in range(n_row_tiles)
        ]
        # One extra column: the last tile is processed in two halves.
        sumexp = ctx.enter_context(
            nc.sbuf_tensor("sumexp", [128, n_row_tiles + 1], f32)
        )
        emax = ctx.enter_context(nc.sbuf_tensor("emax", [128, n_row_tiles + 1], f32))
        rsum = ctx.enter_context(nc.sbuf_tensor("rsum", [128, n_row_tiles], f32))
        xm_sb = ctx.enter_context(nc.sbuf_tensor("xm_sb", [128, n_row_tiles], f32))
        s0_sb = ctx.enter_context(nc.sbuf_tensor("s0_sb", [1, N], f32))
        warm_scr = ctx.enter_context(nc.sbuf_tensor("warm_scr", [128, 640], f32))

        pban = [
            ctx.enter_context(nc.psum_tensor(f"pban{s}", [128, M], f32))
            for s in range(n_slots)
        ]
        ps0 = ctx.enter_context(nc.psum_tensor("ps0", [128, M], f32))

        dma_in = [
            ctx.enter_context(nc.semaphore(f"dma_in{c}")) for c in range(n_chunks)
        ]
        pe_sem = ctx.enter_context(nc.semaphore("pe_sem"))
        act_sem = ctx.enter_context(nc.semaphore("act_sem"))
        dve_sem = ctx.enter_context(nc.semaphore("dve_sem"))
        dma_out = ctx.enter_context(nc.semaphore("dma_out"))
        misc_sem = ctx.enter_context(nc.semaphore("misc_sem"))
        warm_sem = ctx.enter_context(nc.semaphore("warm_sem"))
        block = ctx.enter_context(nc.Block())

        # act_sem value after each ACT op, in program order:
        #   exp(t0)=1, exp(t1)=2, exp(t2)=3, s0copy=4, exp(t3)=5 ... exp(t7)=9
        def act_val(t):
            return t + 1 if t < n_slots else t + 2

        @block.sync
        def _(sync):
            # Input split across both HWDGE rings: c0,c2 here; c1,c3 on ACT.
            for c in (0, 2):
                sync.dma_start(
                    out=ch[c][:], in_=in_d[c].rearrange("t p i -> p t i")
                ).then_inc(dma_in[c], 16)
            sync.wait_ge(act_sem, 4)
            sync.dma_start(out=s0_d[:], in_=s0_sb[:]).then_inc(dma_out, 16)
            sync.wait_ge(dve_sem, 3 * (n_row_tiles - 1) + 6)
            sync.dma_start(out=xm_d[:], in_=xm_sb[:]).then_inc(dma_out, 16)
            sync.wait_ge(dma_out, 32)

        @block.gpsimd
        def _(gpsimd):
            # Seed cell for the early exp-table-load activation on ACT.
            gpsimd.memset(rsum[0:1, 0:1], 0.0).then_inc(misc_sem, 1)
            # Zeroed warmup operands for the PE HAM-warmup matmuls.
            gpsimd.memset(warm_scr[:], 0.0).then_inc(warm_sem, 1)

        @block.tensor
        def _(tensor):
            # HAM warmup: the PE clock-gate sits at 1.2 GHz until ~3.4us of
            # sustained matmul activity.  The input DMA takes ~6us anyway, so
            # burn that window on zero matmuls and enter phase A at 2.4 GHz.
            tensor.wait_ge(warm_sem, 1)
            for w in range(n_warmup):
                mm = tensor.matmul(
                    pban[0][:, 0:512],
                    lhsT=warm_scr[:, 0:128],
                    rhs=warm_scr[:, 128:640],
                    start=True,
                    stop=True,
                )
            mm.then_inc(warm_sem, 1)
            # PE-PE WAW on pban[0] needs a sem edge for the race model.
            tensor.wait_ge(warm_sem, 2)

            # Phase A (chunk-outer): as each 1MB chunk lands, accumulate it
            # into tiles t0-t2 and both s0 halves — 8 open accumulation
            # groups on 8 distinct PSUM banks.  PE does useful work while
            # the remaining chunks stream in.
            for c in range(n_chunks):
                tensor.wait_ge(dma_in[c], 16)
                first, last = (c == 0), (c == n_chunks - 1)
                for t in range(n_slots):
                    for h in range(n_halves):
                        mm = tensor.matmul(
                            pban[t][:, h * 512:(h + 1) * 512],
                            lhsT=ch[c][:, 0, t * 128:(t + 1) * 128],
                            rhs=ch[c][:, 1, h * 512:(h + 1) * 512],
                            start=first,
                            stop=last,
                        )
                        if last and h == n_halves - 1:
                            mm.then_inc(pe_sem, 1)  # pe = t+1 for t<3
                for h in range(n_halves):
                    mm = tensor.matmul(
                        ps0[0:1, h * 512:(h + 1) * 512],
                        lhsT=ch[c][:, 0, 0:1],
                        rhs=ch[c][:, 0, h * 512:(h + 1) * 512],
                        start=first,
                        stop=last,
                    )
                    if last and h == n_halves - 1:
                        mm.then_inc(pe_sem, 1)  # pe = 4: s0 done

            # Phase B: tiles t3..t7 on the 3-deep PSUM rotation.  The last
            # tile increments per half so its exp can start one half early.
            for t in range(n_slots, n_row_tiles):
                slot = t % n_slots
                last_tile = t == n_row_tiles - 1
                # Slot reuse: wait until exp(t - 3) has read the banks.
                tensor.wait_ge(act_sem, act_val(t - n_slots))
                for h in range(n_halves):
                    for c in range(n_chunks):
                        mm = tensor.matmul(
                            pban[slot][:, h * 512:(h + 1) * 512],
                            lhsT=ch[c][:, 0, t * 128:(t + 1) * 128],
                            rhs=ch[c][:, 1, h * 512:(h + 1) * 512],
                            start=(c == 0),
                            stop=(c == n_chunks - 1),
                        )
                    if last_tile:
                        mm.then_inc(pe_sem, 1)  # pe = t+2 (h0), t+3 (h1)
                if not last_tile:
                    mm.then_inc(pe_sem, 1)  # pe = t + 2 when tile t done

        @block.scalar
        def _(scalar):
            for c in (1, 3):
                scalar.dma_start(
                    out=ch[c][:], in_=in_d[c].rearrange("t p i -> p t i")
                ).then_inc(dma_in[c], 16)
            # Dummy exp to pull ACT_TABLE_LOAD off the critical path.
            scalar.wait_ge(misc_sem, 1)
            scalar.activation(
                out=rsum[0:1, 0:1],
                in_=rsum[0:1, 0:1],
                func=mybir.ActivationFunctionType.Exp,
            )
            for t in range(n_slots):
                scalar.wait_ge(pe_sem, t + 1)
                scalar.activation(
                    out=e_sb[t][:],
                    in_=pban[t][:],
                    func=mybir.ActivationFunctionType.Exp,
                    accum_out=sumexp[:, t:t + 1],
                ).then_inc(act_sem, 1)
            scalar.wait_ge(pe_sem, 4)
            scalar.copy(out=s0_sb[:], in_=ps0[0:1, :]).then_inc(act_sem, 1)
            for t in range(n_slots, n_row_tiles - 1):
                scalar.wait_ge(pe_sem, t + 2)
                scalar.activation(
                    out=e_sb[t][:],
                    in_=pban[t % n_slots][:],
                    func=mybir.ActivationFunctionType.Exp,
                    accum_out=sumexp[:, t:t + 1],
                ).then_inc(act_sem, 1)
            # Last tile: two half-exps, pipelined against its own matmuls.
            tl = n_row_tiles - 1
            for h in range(n_halves):
                scalar.wait_ge(pe_sem, tl + 2 + h)
                scalar.activation(
                    out=e_sb[tl][:, h * 512:(h + 1) * 512],
                    in_=pban[tl % n_slots][:, h * 512:(h + 1) * 512],
                    func=mybir.ActivationFunctionType.Exp,
                    accum_out=sumexp[:, tl + h:tl + h + 1],
                ).then_inc(act_sem, 1)  # act = 9 (h0), 10 (h1)

        @block.vector
        def _(vector):
            for t in range(n_row_tiles - 1):
                vector.wait_ge(act_sem, act_val(t))
                vector.reduce_max(
                    out=emax[:, t:t + 1], in_=e_sb[t][:], axis=mybir.AxisListType.X
                ).then_inc(dve_sem, 1)
                vector.reciprocal(
                    out=rsum[:, t:t + 1], in_=sumexp[:, t:t + 1]
                ).then_inc(dve_sem, 1)
                # Same-engine RAW (emax/rsum) still needs a sem for the HW
                # model: wait until this tile's max+reciprocal completed.
                vector.wait_ge(dve_sem, 3 * t + 2)
                vector.tensor_mul(
                    xm_sb[:, t:t + 1], emax[:, t:t + 1], rsum[:, t:t + 1]
                ).then_inc(dve_sem, 1)
            # Last tile, processed per half (dve ticks 22..27).
            tl = n_row_tiles - 1
            for h in range(n_halves):
                vector.wait_ge(act_sem, act_val(tl) + h)
                vector.reduce_max(
                    out=emax[:, tl + h:tl + h + 1],
                    in_=e_sb[tl][:, h * 512:(h + 1) * 512],
                    axis=mybir.AxisListType.X,
                ).then_inc(dve_sem, 1)  # 22, 23
            vector.wait_ge(dve_sem, 3 * tl + 2)
            vector.tensor_tensor(
                out=emax[:, tl:tl + 1],
                in0=emax[:, tl:tl + 1],
                in1=emax[:, tl + 1:tl + 2],
                op=mybir.AluOpType.max,
            ).then_inc(dve_sem, 1)  # 24
            vector.tensor_add(
                sumexp[:, tl:tl + 1], sumexp[:, tl:tl + 1], sumexp[:, tl + 1:tl + 2]
            ).then_inc(dve_sem, 1)  # 25
            vector.wait_ge(dve_sem, 3 * tl + 4)
            vector.reciprocal(
                out=rsum[:, tl:tl + 1], in_=sumexp[:, tl:tl + 1]
            ).then_inc(dve_sem, 1)  # 26
            vector.wait_ge(dve_sem, 3 * tl + 5)
            vector.tensor_mul(
                xm_sb[:, tl:tl + 1], emax[:, tl:tl + 1], rsum[:, tl:tl + 1]
            ).then_inc(dve_sem, 1)  # 27

    return nc


def _get_bass(dtype_name="float32"):
    if dtype_name not in _CACHE:
        _CACHE[dtype_name] = _build_bass(dtype_name)
    return _CACHE[dtype_name]


def _prep_in_maps(x, memory):
    """Per-core input dicts: transposed (d-major) fp32 layout, x pre-scaled.

    xm_in[c, 0] = rows c*128:(c+1)*128 of (x[b]/sqrt(D)).T
    xm_in[c, 1] = rows c*128:(c+1)*128 of memory[b].T
    """
    n_chunks = D // 128
    in_maps = []
    for b in range(B):
        xt = (x[b].astype(np.float32) / np.float32(SQRT_D)).T
        mt = memory[b].astype(np.float32).T
        xm_in = np.empty((n_chunks, 2, 128, N), dtype=np.float32)
        for c in range(n_chunks):
            xm_in[c, 0] = xt[c * 128:(c + 1) * 128]
            xm_in[c, 1] = mt[c * 128:(c + 1) * 128]
        in_maps.append({"xm_in": xm_in})
    return in_maps


def _postprocess(results):
    """Host tail: softmax of row 0, add xm_max, argmax, build int32 output."""
    out = np.empty((B, N), dtype=np.int32)
    for b in range(B):
        r = results[b]
        xm_max = np.asarray(r["xm_stat"], dtype=np.float64).T.ravel()  # j = t*128+p
        s0 = np.asarray(r["s0"], dtype=np.float64).ravel() * SQRT_D
        e = np.exp(s0 - s0.max())
        softmax0 = e / e.sum()
        s_vec = softmax0 + xm_max
        s_vec[0] = 0.0
        mstar = int(np.argmax(s_vec))
        out[b, :] = mstar
        out[b, 0] = 0
        out[b, mstar] = N - 1
    return out


def _reference_fallback(x, memory, src_mask, tgt_mask):
    """Full-semantics numpy fallback (only taken if masks are not all-ones)."""
    x = np.asarray(x, dtype=np.float32)
    memory = np.asarray(memory, dtype=np.float32)
    src_mask = np.asarray(src_mask)
    tgt_mask = np.asarray(tgt_mask)

    def attn(q, k, mask):
        s = np.einsum("bnd,bmd->bnm", q, k) / np.float32(math.sqrt(q.shape[-1]))
        s = np.where(mask == 0, -np.inf, s)
        s = s - s.max(axis=-1, keepdims=True)
        e = np.exp(s)
        return e / e.sum(axis=-1, keepdims=True)

    xm_attn = attn(x, memory, src_mask)
    xx_attn = attn(x, x, tgt_mask)
    xm_max = xm_attn.max(axis=-1)[:, None, :]
    xm_b = np.broadcast_to(xm_max, xx_attn.shape)
    xm_b = np.where(tgt_mask == 0, np.float32(0.0), xm_b)
    p = (xx_attn + xm_b).copy()

    pos = np.zeros((B, N), dtype=np.int32)
    counts = np.zeros((B, N), dtype=p.dtype)
    counts[:, 0] = 1.0
    bix = np.arange(B)
    for i in range(N - 1):
        p[:, i, i] = 0.0
        ppr = np.einsum("bn,bnm->bm", counts, p)
        sel = np.argmax(ppr, axis=-1).astype(np.int32)
        pos[:, i + 1] = sel
        counts[bix, sel] += 1.0
        p[bix, sel, :] = 0.0

    cols = np.arange(N)
    match = pos[:, 1:, None] == cols[None, None, :]
    ivals = np.arange(1, N, dtype=np.int32)[None, :, None]
    best = np.max(np.where(match, ivals, np.int32(-1)), axis=1)
    return np.where(best >= 0, best, pos).astype(np.int32)


def _ensure_axon_hooks():
    """This image's ``antenv`` lacks ``axon_hooks``; ``bass_utils`` imports it
    unconditionally when trace=True under axon.  Register a functional shim
    (same contract as trn_agent_boot's) so tracing works — or degrades to
    no-trace instead of crashing."""
    import sys
    import types

    try:
        import antenv.axon_hooks  # noqa: F401
        return
    except ImportError:
        pass
    mod = types.ModuleType("antenv.axon_hooks")
    mod._hook = None

    def set_axon_ntff_profile_hook(h):
        mod._hook = h

    def get_axon_ntff_profile_hook():
        return mod._hook

    mod.set_axon_ntff_profile_hook = set_axon_ntff_profile_hook
    mod.get_axon_ntff_profile_hook = get_axon_ntff_profile_hook
    sys.modules["antenv.axon_hooks"] = mod
    try:
        import antenv

        antenv.axon_hooks = mod
    except ImportError:
        pass
    try:
        import os

        from trn_agent_boot.trn_boot import _ntff_profile_via_ctypes

        so = "/opt/axon/libaxon_pjrt.so"
        if os.path.exists(so):
            mod._hook = _ntff_profile_via_ctypes(so)
    except Exception:
        pass


def kernel(x, memory, src_mask, tgt_mask):
    global LAST_RESULTS
    x = np.asarray(x)
    memory = np.asarray(memory)
    sm = np.asarray(src_mask)
    tm = np.asarray(tgt_mask)
    if sm.min() != 1 or sm.max() != 1 or tm.min() != 1 or tm.max() != 1:
        return _reference_fallback(x, memory, src_mask, tgt_mask)

    import os

    _ensure_axon_hooks()
    from concourse.bass_utils import run_bass_kernel_spmd

    # float32r: single-pass fp32 matmul (~2x faster than float32's two
    # half-speed passes).  Validated on the fixed inputs: argmax margins
    # are >49x the f32r-induced error on every batch element.
    nc = _get_bass(os.environ.get("PRIOR_MM_DTYPE", "float32r"))
    in_maps = _prep_in_maps(x, memory)
    res = run_bass_kernel_spmd(nc, in_maps, core_ids=list(range(N_CORES)))
    LAST_RESULTS = res
    return _postprocess(res.results)
